# revision 10
# baseline (speedup 1.0000x reference)
"""CharLSTM Trainium2 kernel.

Model: tokens [512, 512] -> emb gather -> xw = x @ W_ih.T + biases
-> 512-step LSTM recurrence -> h_last @ W_cls.T + b_cls -> [512, 256] logits.

Strategy:
  * Data-parallel over batch: 8 cores x 64 sequences each.  Weights replicated.
  * Per core, state kept transposed: hT/mT = [128 (hid), 64 (batch)] split into
    S=2 pipelined sub-blocks of 32 so PE/ACT/DVE overlap across the serial
    T=512 chain.
  * The embedding + input projection + biases are fused into one 256x512 table
    (table = emb @ W_ih_r.T + b) computed on device once, kept in SBUF, and
    gathered per 8-step chunk with gpsimd dma_gather(transpose=True), which
    lands xwT directly in the [gate-dims, tokens] orientation the recurrence
    needs.  This avoids materializing the 64 MB xw tensor in HBM.
  * Gate columns are reordered to [f, g, i, o] and the i/f/o pre-activations
    are halved on the host so sigmoid(z) = (tanh(z/2)+1)/2 comes out of ONE
    Tanh activation over [f,g,i]; the o-gate gets its own small Tanh (it is
    off the critical path until the very end of the step).
  * The c-state is kept as m = 2c.  The per-step serial chain is then:
      PE gates -> ACT tanh[f,g,i] -> DVE u1=(tf+1)*m -> DVE m'=0.5*u1+u2
      -> DVE h' (custom fused op) -> PE next gates
    where u2=(ti+1)*tg runs off-chain.  tanh(c') for the hidden update is a
    clamped cubic evaluated INSIDE a custom 8-stage DVE instruction
    (LSTM_HGATE_ANT) fused with the output gate:
      h~ = (to+1) * y*(1 + RC*y^2),  y = clamp(m', +-M0)
    The cubic's leading coefficient A1 and the 1/2 factors are folded into
    W_hh / W_cls on the host (stored hidden state is h~ = 2h/A1).  This
    removes the second Activation-engine visit (tanh(c)) and one DVE stage
    from the serial chain: modeled step time drops ~2030ns -> ~1400ns.
"""

import os
import sys
from contextlib import ExitStack

import numpy as np

for _p in ("/opt/trn_rl_repo", "/opt/pypackages"):
    if _p not in sys.path and os.path.isdir(_p):
        sys.path.append(_p)

VOCAB, EMB, HID = 256, 32, 128
B, T = 512, 512
N_CORES = 8
BC = B // N_CORES  # 64 sequences per core
S = 2              # pipelined sub-blocks per core
BS = BC // S       # 32
G4 = 4 * HID       # 512 gate dims

# clamped-cubic tanh(c) approximation (c tracked as m = 2c):
#   tanh(m/2) ~= A1 * y * (1 + RC*y^2),  y = clamp(m, +-M0)
# Fit of tanh(m/2) on [0, M0]; end-to-end logit rel-err vs exact ~4e-3.
M0 = 1.6
A1 = 0.4951799
RC = -0.0649799

_HOP_CACHE = {}


def _register_custom_ops():
    """Register the fused output-gate custom DVE op (idempotent)."""
    if "op" in _HOP_CACHE:
        return _HOP_CACHE["op"]
    import concourse.dve_ops as dve_ops

    for op in dve_ops.OPS:
        if op.name == "LSTM_HGATE_ANT":
            _HOP_CACHE["op"] = op
            return op

    from concourse.dve_spec import (
        C0,
        C1,
        One,
        Spec,
        Src0,
        Src1,
        Zero,
        _has_src1,
        lower,
        maxx,
        minn,
        sq,
    )
    from concourse.dve_uop import DveOpSpec

    y = maxx(minn(Src1, C0), Zero - C0)
    body = (Src0 + One) * (y * (sq(y) * C1 + One))

    def _ref(in0, in1, s0, s1, imm2):
        yy = np.clip(in1.astype(np.float32), -s0, s0)
        return (
            (in0.astype(np.float32) + 1.0) * (yy * (yy * yy * s1 + 1.0))
        ).astype(np.float32)

    spec = Spec(body=body, reference=_ref)
    row = dve_ops._CUSTOM_DVE_ROW_BASE + len(dve_ops.OPS)
    shas = {}
    for ver in ("v3", "v4"):
        try:
            u = lower(spec, ver=ver)
            shas[ver] = DveOpSpec(
                name="LSTM_HGATE_ANT", opcode=row, uops=u, rd1_en=_has_src1(spec)
            ).sha(ver)
        except Exception:
            pass
    op = dve_ops.DveOp("LSTM_HGATE_ANT", spec, subdim=False, uops_sha=shas)
    dve_ops.OPS.append(op)
    dve_ops.CUSTOM_DVE_SPECS[op.name] = spec
    dve_ops._SUB_OPCODE_FOR_NAME[op.name] = row
    _HOP_CACHE["op"] = op
    return op


def build_kernel(t_steps=T, ch=8, debug=False):
    """Build + compile the per-core SPMD program. Returns the Bacc object."""
    import concourse.bacc as bacc
    import concourse.bass as bass
    import concourse.mybir as mybir
    import concourse.tile as tile

    hop = _register_custom_ops()

    dt = mybir.dt
    AF = mybir.ActivationFunctionType
    Alu = mybir.AluOpType
    f32, f16, i16 = dt.float32, dt.float16, dt.int16

    assert t_steps % ch == 0
    nidx_ch = ch * BC          # gathered tokens per chunk
    assert nidx_ch % 128 == 0

    nc = bacc.Bacc(
        "TRN2",
        target_bir_lowering=False,
        debug=debug,
        num_devices=N_CORES,
    )

    # ---- I/O ----
    embT_d = nc.dram_tensor("embT", [EMB + 1, VOCAB], f32, kind="ExternalInput")
    wih_d = nc.dram_tensor("wih", [EMB + 1, G4], f32, kind="ExternalInput")
    whh_d = nc.dram_tensor("whh", [HID, G4], f16, kind="ExternalInput")
    id_d = nc.dram_tensor("ident", [HID, HID], f16, kind="ExternalInput")
    wcls_d = nc.dram_tensor("wcls", [HID, VOCAB], f16, kind="ExternalInput")
    bcls_d = nc.dram_tensor("bcls", [1, VOCAB], f16, kind="ExternalInput")
    idxs_d = nc.dram_tensor(
        "idxs", [128, t_steps * BC // 16], i16, kind="ExternalInput"
    )
    out_d = nc.dram_tensor("out", [BC, VOCAB], f32, kind="ExternalOutput")

    with tile.TileContext(nc) as tc, ExitStack() as ctx:
        const = ctx.enter_context(tc.tile_pool(name="const", bufs=1))
        ptab = ctx.enter_context(
            tc.tile_pool(name="ptab", bufs=1, space=bass.MemorySpace.PSUM)
        )
        psg = ctx.enter_context(
            tc.tile_pool(name="psg", bufs=3, space=bass.MemorySpace.PSUM)
        )
        gpool = ctx.enter_context(tc.tile_pool(name="gpool", bufs=3))
        spool = ctx.enter_context(tc.tile_pool(name="spool", bufs=8))

        # ---- load constants ----
        embT_sb = const.tile([EMB + 1, VOCAB], f32, tag="embT")
        nc.sync.dma_start(embT_sb[:], embT_d[:])
        wih_sb = const.tile([EMB + 1, G4], f32, tag="wih")
        nc.sync.dma_start(wih_sb[:], wih_d[:])
        whh_sb = const.tile([HID, G4], f16, tag="whh")
        nc.sync.dma_start(whh_sb[:], whh_d[:])
        id_sb = const.tile([HID, HID], f16, tag="ident")
        nc.sync.dma_start(id_sb[:], id_d[:])
        wcls_sb = const.tile([HID, VOCAB], f16, tag="wcls")
        nc.sync.dma_start(wcls_sb[:], wcls_d[:])
        bcls_sb = const.tile([1, VOCAB], f16, tag="bcls")
        nc.sync.dma_start(bcls_sb[:], bcls_d[:])
        idx_sb = const.tile([128, t_steps * BC // 16], i16, tag="idxs")
        nc.sync.dma_start(idx_sb[:], idxs_d[:])

        # ---- build the fused token table in SBUF ----
        # table row (vocab v) holds [f | g | i | o] pre-activations incl. both
        # biases (i/f/o halved for the sigmoid-via-tanh trick).  SBUF layout is
        # rank-packed for dma_gather's SBUF-source mode: partition p, sub-row r
        # (of 2) = vocab row 2p + r; sub-row r occupies fp16 columns
        # [r*512, (r+1)*512).  The host permutes embT's columns so chunk r's
        # matmul directly produces the vocab rows {2j + r}.
        table = const.tile([128, 2 * G4], f16, tag="table")
        for r in range(2):
            pt = ptab.tile([128, G4], f32, tag="ptab")
            nc.tensor.matmul(
                pt[:],
                embT_sb[:, r * 128 : (r + 1) * 128],
                wih_sb[:],
                start=True,
                stop=True,
            )
            nc.vector.tensor_copy(table[:, r * G4 : (r + 1) * G4], pt[:])

        ones_sb = const.tile([1, BC], f16, tag="ones")
        nc.vector.memset(ones_sb[:], 1.0)

        # ---- state ----
        hT = []
        mT = []
        for s in range(S):
            h = const.tile([HID, BS], f16, tag=f"h{s}")
            m = const.tile([HID, BS], f32, tag=f"m{s}")
            nc.vector.memset(h[:], 0.0)
            nc.vector.memset(m[:], 0.0)
            hT.append(h)
            mT.append(m)

        # ---- recurrence ----
        n_chunks = t_steps // ch
        xw_tiles = {}

        def emit_gather(c):
            xw = gpool.tile([128, 4, nidx_ch], f16, tag="xw")
            nc.gpsimd.dma_gather(
                xw[:],
                table[:],
                idx_sb[:, c * (nidx_ch // 16) : (c + 1) * (nidx_ch // 16)],
                nidx_ch,
                nidx_ch,
                G4,
                transpose=True,
                single_packet=False,
                sbuf_tokens_per_rank=128,
                sbuf_free_dim_per_rank=G4 * 2,  # 1024 B: one full row per rank stripe
            )
            xw_tiles[c] = xw

        emit_gather(0)
        for c in range(n_chunks):
            if c + 1 < n_chunks:
                emit_gather(c + 1)
            xw = xw_tiles.pop(c)
            for k in range(ch):
                toff = k * BC
                ps_cur = []
                # 5-matmul PSUM group per sub-block: gate matmuls (first one
                # start=True zeroes the full-bank tile) then the xw injection
                # last.  Keeping the xw matmul inside the group makes the
                # scheduler place it with the gates instead of deferring it
                # behind a coarse Activation-semaphore wait.
                for s in range(S):
                    ps_full = psg.tile([128, 512], f32, tag=f"ps{s}")
                    ps = ps_full[:, 0 : 4 * BS]
                    ps_cur.append(ps)
                    for gb in range(4):
                        nc.tensor.matmul(
                            ps[:, gb * BS : (gb + 1) * BS],
                            whh_sb[:, gb * HID : (gb + 1) * HID],
                            hT[s][:],
                            start=(gb == 0),
                            stop=False,
                            skip_group_check=True,
                        )
                    nc.tensor.matmul(
                        ps[:],
                        id_sb[:],
                        xw[:, :, toff + s * BS : toff + (s + 1) * BS],
                        start=False,
                        stop=True,
                        skip_group_check=True,
                    )
                # elementwise cell per sub-block
                for s in range(S):
                    ps = ps_cur[s]
                    # ONE tanh over [f,g,i]: tf = tanh(zf/2) etc (host-halved)
                    sg3 = spool.tile([128, 3 * BS], f16, tag=f"sg3{s}")
                    nc.scalar.activation(sg3[:], ps[:, 0 : 3 * BS], AF.Tanh)
                    # o-gate tanh: only needed by the last DVE op of the step
                    to = spool.tile([128, BS], f16, tag=f"to{s}")
                    nc.scalar.activation(to[:], ps[:, 3 * BS : 4 * BS], AF.Tanh)
                    # u1 = (tf+1)*m  = 2f * m          (critical path)
                    u1 = spool.tile([128, BS], f32, tag=f"u1{s}")
                    nc.vector.scalar_tensor_tensor(
                        u1[:], sg3[:, 0:BS], 1.0, mT[s][:], Alu.add, Alu.mult
                    )
                    # u2 = (ti+1)*tg = 2i * g          (off critical path)
                    u2 = spool.tile([128, BS], f16, tag=f"u2{s}")
                    nc.vector.scalar_tensor_tensor(
                        u2[:], sg3[:, 2 * BS : 3 * BS], 1.0,
                        sg3[:, BS : 2 * BS], Alu.add, Alu.mult,
                    )
                    # m' = 0.5*u1 + u2  (= 2c')
                    nc.vector.scalar_tensor_tensor(
                        mT[s][:], u1[:], 0.5, u2[:], Alu.mult, Alu.add
                    )
                    # h~' = (to+1) * y*(1 + RC*y^2), y = clamp(m', +-M0)
                    nc.vector._custom_dve(
                        hop, out=hT[s][:], in0=to[:], in1=mT[s][:],
                        s0=float(M0), s1=float(RC),
                    )

        # ---- classifier ----
        hall = spool.tile([HID, BC], f16, tag="hall")
        for s in range(S):
            nc.vector.tensor_copy(hall[:, s * BS : (s + 1) * BS], hT[s][:])
        pc = ptab.tile([BC, VOCAB], f32, tag="pcls")
        nc.tensor.matmul(
            pc[:],
            hall[:],
            wcls_sb[:],
            start=True,
            stop=False,
            skip_group_check=True,
        )
        nc.tensor.matmul(
            pc[:],
            ones_sb[:],
            bcls_sb[:],
            start=False,
            stop=True,
            skip_group_check=True,
        )
        out_sb = spool.tile([BC, VOCAB], f32, tag="out")
        nc.vector.tensor_copy(out_sb[:], pc[:])
        nc.sync.dma_start(out_d[:], out_sb[:])

    nc.compile()
    return nc


def prep_inputs(inputs, emb, W_ih, W_hh, b_ih, b_hh, W_cls, b_cls, t_steps=T):
    """Host-side input marshaling: gate reorder [f,g,i,o], sigmoid-via-tanh
    pre-halving, h~ = 2h/A1 state-scale folding, transposes, vocab interleave
    permutation, and per-core token index wrap."""
    # torch gate order in rows: i [0:H], f [H:2H], g [2H:3H], o [3H:4H]
    perm = np.concatenate(
        [np.arange(128, 256), np.arange(256, 384), np.arange(0, 128),
         np.arange(384, 512)]
    )  # -> [f, g, i, o]
    # i/f/o pre-activations halved (sigmoid(z) = (tanh(z/2)+1)/2); g unscaled.
    blk = np.concatenate(
        [np.full(HID, 0.5), np.full(HID, 1.0), np.full(HID, 0.5),
         np.full(HID, 0.5)]
    ).astype(np.float32)

    Wih_r = np.asarray(W_ih, np.float32)[perm] * blk[:, None]
    bias_r = (np.asarray(b_ih, np.float32) + np.asarray(b_hh, np.float32))[perm] * blk
    # W_hh additionally absorbs the h~ = 2h/A1 state scale: h = 0.5*A1*h~.
    Whh_r = np.asarray(W_hh, np.float32)[perm] * blk[:, None] * (0.5 * A1)

    embT_perm = np.concatenate(
        [np.asarray(emb, np.float32).T, np.ones((1, VOCAB), np.float32)], axis=0
    )  # [33, 256]; chunk r cols = vocab [128r, 128r+128)
    wih_aug = np.concatenate([Wih_r.T, bias_r[None, :]], axis=0)  # [33, 512]
    wih_aug = np.ascontiguousarray(wih_aug)

    common = {
        "embT": embT_perm.astype(np.float32),
        "wih": wih_aug.astype(np.float32),
        "whh": np.ascontiguousarray(Whh_r.T).astype(np.float16),
        "ident": np.eye(HID, dtype=np.float16),
        "wcls": np.ascontiguousarray(
            (0.5 * A1) * np.asarray(W_cls, np.float32).T
        ).astype(np.float16),
        "bcls": np.asarray(b_cls, np.float32)[None, :].astype(np.float16),
    }

    tok = np.asarray(inputs)
    in_maps = []
    for cidx in range(N_CORES):
        tc_ = tok[cidx * BC : (cidx + 1) * BC, :t_steps]  # [64, t]
        flat = tc_.T.reshape(-1).astype(np.int16)  # t-major: idx j = t*64 + b
        wrapped = flat.reshape(-1, 16).T  # [16, n/16]; idx j at [j%16, j//16]
        idxs = np.ascontiguousarray(np.tile(wrapped, (8, 1)))  # [128, n/16]
        m = dict(common)
        m["idxs"] = idxs
        in_maps.append(m)
    return in_maps


_NC_CACHE = {}


def kernel(inputs, emb, W_ih, W_hh, b_ih, b_hh, W_cls, b_cls):
    import concourse.bass_utils as bass_utils

    if "nc" not in _NC_CACHE:
        _NC_CACHE["nc"] = build_kernel()
    nc = _NC_CACHE["nc"]
    in_maps = prep_inputs(inputs, emb, W_ih, W_hh, b_ih, b_hh, W_cls, b_cls)
    res = bass_utils.run_bass_kernel_spmd(
        nc, in_maps, core_ids=list(range(N_CORES))
    )
    out = np.concatenate([r["out"] for r in res.results], axis=0)
    return np.ascontiguousarray(out.astype(np.float32))


# revision 21
# speedup vs baseline: 1.0035x; 1.0035x over previous
"""CharLSTM Trainium2 kernel.

Model: tokens [512, 512] -> emb gather -> xw = x @ W_ih.T + biases
-> 512-step LSTM recurrence -> h_last @ W_cls.T + b_cls -> [512, 256] logits.

Strategy:
  * Data-parallel over batch: 8 cores x 64 sequences each.  Weights replicated.
  * Per core, state kept transposed: hT/mT = [128 (hid), 64 (batch)] split into
    S=2 pipelined sub-blocks of 32 so PE/ACT/DVE overlap across the serial
    T=512 chain.
  * The embedding + input projection + biases are fused into one 256x512 table
    (table = emb @ W_ih_r.T + b) computed on device once, kept in SBUF, and
    gathered per 8-step chunk with gpsimd dma_gather(transpose=True), which
    lands xwT directly in the [gate-dims, tokens] orientation the recurrence
    needs.  This avoids materializing the 64 MB xw tensor in HBM.
  * Gate columns are reordered to [f, g, i, o] and the i/f/o pre-activations
    are halved on the host so sigmoid(z) = (tanh(z/2)+1)/2 comes out of ONE
    Tanh activation over [f,g,i]; the o-gate gets its own small Tanh (it is
    off the critical path until the very end of the step).
  * The c-state is kept as m = 2c.  The per-step serial chain is then:
      PE gates -> ACT tanh[f,g,i] -> DVE u1=(tf+1)*m -> DVE m'=0.5*u1+u2
      -> DVE h' (custom fused op) -> PE next gates
    where u2=(ti+1)*tg runs off-chain.  tanh(c') for the hidden update is a
    clamped cubic evaluated INSIDE a custom 8-stage DVE instruction
    (LSTM_HGATE_ANT) fused with the output gate:
      h~ = (to+1) * y*(1 + RC*y^2),  y = clamp(m', +-M0)
    The cubic's leading coefficient A1 and the 1/2 factors are folded into
    W_hh / W_cls on the host (stored hidden state is h~ = 2h/A1).  This
    removes the second Activation-engine visit (tanh(c)) and one DVE stage
    from the serial chain: modeled step time drops ~2030ns -> ~1400ns.
"""

import os
import sys
from contextlib import ExitStack

import numpy as np

for _p in ("/opt/trn_rl_repo", "/opt/pypackages"):
    if _p not in sys.path and os.path.isdir(_p):
        sys.path.append(_p)

VOCAB, EMB, HID = 256, 32, 128
B, T = 512, 512
N_CORES = 8
BC = B // N_CORES  # 64 sequences per core
BSL = [32, 32]     # pipelined sub-block sizes per core (sum = BC)
S = len(BSL)
BOFF = [sum(BSL[:i]) for i in range(S)]  # batch offsets of each sub-block
PSG_BUFS = 2       # PSUM tiles in flight per sub-block
G4 = 4 * HID       # 512 gate dims

# clamped-cubic tanh(c) approximation (c tracked as m = 2c):
#   tanh(m/2) ~= A1 * y * (1 + RC*y^2),  y = clamp(m, +-M0)
# Fit of tanh(m/2) on [0, M0]; end-to-end logit rel-err vs exact ~4e-3.
M0 = 1.6
A1 = 0.4951799
RC = -0.0649799

# o-gate: tanh(zo/2) ~= P1O*y + P3O*y^3, y = clamp(zo/2, +-X0O) (cubic sigma)
X0O = 1.1
P1O = 0.972540
P3O = -0.211690

_HOP_CACHE = {}


def _register_one(name, spec):
    """Register one custom DVE op (idempotent); returns the DveOp."""
    import concourse.dve_ops as dve_ops
    from concourse.dve_spec import _has_src1, lower
    from concourse.dve_uop import DveOpSpec

    for op in dve_ops.OPS:
        if op.name == name:
            return op
    row = dve_ops._CUSTOM_DVE_ROW_BASE + len(dve_ops.OPS)
    shas = {}
    for ver in ("v3", "v4"):
        try:
            u = lower(spec, ver=ver)
            shas[ver] = DveOpSpec(
                name=name, opcode=row, uops=u, rd1_en=_has_src1(spec)
            ).sha(ver)
        except Exception:
            pass
    op = dve_ops.DveOp(name, spec, subdim=False, uops_sha=shas)
    dve_ops.OPS.append(op)
    dve_ops.CUSTOM_DVE_SPECS[op.name] = spec
    dve_ops._SUB_OPCODE_FOR_NAME[op.name] = row
    return op


def _register_custom_ops():
    """Register the fused LSTM custom DVE ops (idempotent)."""
    if "ops" in _HOP_CACHE:
        return _HOP_CACHE["ops"]
    from concourse.dve_spec import (
        C0,
        C1,
        C2,
        One,
        Spec,
        Src0,
        Src1,
        Zero,
        maxx,
        minn,
        sq,
    )

    # h~ = (to + 1) * y*(1 + C1*y^2), y = clamp(m', +-C0)
    y = maxx(minn(Src1, C0), Zero - C0)
    hbody = (Src0 + One) * (y * (sq(y) * C1 + One))

    def _href(in0, in1, s0, s1, imm2):
        yy = np.clip(in1.astype(np.float32), -s0, s0)
        return (
            (in0.astype(np.float32) + 1.0) * (yy * (yy * yy * s1 + 1.0))
        ).astype(np.float32)

    hop = _register_one("LSTM_HGATE_ANT", Spec(body=hbody, reference=_href))

    # to = C2*y + C1*y^3, y = clamp(zo, +-C0): cubic tanh for the o-gate,
    # read straight from PSUM (zo is pre-halved on the host)
    yo = maxx(minn(Src0, C0), Zero - C0)
    tbody = (sq(yo) * C1 + C2) * yo

    def _tref(in0, in1, s0, s1, imm2):
        yy = np.clip(in0.astype(np.float32), -s0, s0)
        return ((yy * yy * s1 + imm2) * yy).astype(np.float32)

    top = _register_one("LSTM_TO_ANT", Spec(body=tbody, reference=_tref))
    _HOP_CACHE["ops"] = (hop, top)
    return hop, top


def build_kernel(t_steps=T, ch=2, debug=False):
    """Build + compile the per-core SPMD program. Returns the Bacc object."""
    import concourse.bacc as bacc
    import concourse.bass as bass
    import concourse.mybir as mybir
    import concourse.tile as tile

    hop, top = _register_custom_ops()

    dt = mybir.dt
    AF = mybir.ActivationFunctionType
    Alu = mybir.AluOpType
    f32, f16, i16 = dt.float32, dt.float16, dt.int16

    assert t_steps % ch == 0
    nidx_ch = ch * BC          # gathered tokens per chunk
    assert nidx_ch % 128 == 0

    nc = bacc.Bacc(
        "TRN2",
        target_bir_lowering=False,
        debug=debug,
        num_devices=N_CORES,
    )

    # ---- I/O ----
    table_d = nc.dram_tensor("table", [128, 2 * G4], f16, kind="ExternalInput")
    whh_d = nc.dram_tensor("whh", [HID, G4], f16, kind="ExternalInput")
    id_d = nc.dram_tensor("ident", [HID, HID], f16, kind="ExternalInput")
    wcls_d = nc.dram_tensor("wcls", [HID, VOCAB], f16, kind="ExternalInput")
    bcls_d = nc.dram_tensor("bcls", [1, VOCAB], f16, kind="ExternalInput")
    idxs_d = nc.dram_tensor(
        "idxs", [128, t_steps * BC // 16], i16, kind="ExternalInput"
    )
    out_d = nc.dram_tensor("out", [BC, VOCAB], f32, kind="ExternalOutput")

    with tile.TileContext(nc) as tc, ExitStack() as ctx:
        const = ctx.enter_context(tc.tile_pool(name="const", bufs=1))
        ptab = ctx.enter_context(
            tc.tile_pool(name="ptab", bufs=1, space=bass.MemorySpace.PSUM)
        )
        psg = ctx.enter_context(
            tc.tile_pool(name="psg", bufs=PSG_BUFS, space=bass.MemorySpace.PSUM)
        )
        gpool = ctx.enter_context(tc.tile_pool(name="gpool", bufs=3))
        spool = ctx.enter_context(tc.tile_pool(name="spool", bufs=8))

        # ---- load constants ----
        whh_sb = const.tile([HID, G4], f16, tag="whh")
        nc.sync.dma_start(whh_sb[:], whh_d[:])
        id_sb = const.tile([HID, HID], f16, tag="ident")
        nc.sync.dma_start(id_sb[:], id_d[:])
        wcls_sb = const.tile([HID, VOCAB], f16, tag="wcls")
        nc.sync.dma_start(wcls_sb[:], wcls_d[:])
        bcls_sb = const.tile([1, VOCAB], f16, tag="bcls")
        nc.sync.dma_start(bcls_sb[:], bcls_d[:])
        idx_sb = const.tile([128, t_steps * BC // 16], i16, tag="idxs")
        nc.sync.dma_start(idx_sb[:], idxs_d[:])

        # ---- fused token table (built on host from emb/W_ih/biases) ----
        # table row (vocab v) holds [f | g | i | o] pre-activations incl. both
        # biases (i/f/o halved for the sigmoid-via-tanh trick).  SBUF layout is
        # rank-packed for dma_gather's SBUF-source mode: vocab row v lives at
        # partition v %% 128, fp16 columns [(v // 128)*512, (v // 128 + 1)*512).
        table = const.tile([128, 2 * G4], f16, tag="table")
        nc.sync.dma_start(table[:], table_d[:])

        ones_sb = const.tile([1, BC], f16, tag="ones")
        nc.vector.memset(ones_sb[:], 1.0)

        # ---- state ----
        hT = []
        mT = []
        for s in range(S):
            h = const.tile([HID, BSL[s]], f16, tag=f"h{s}")
            m = const.tile([HID, BSL[s]], f32, tag=f"m{s}")
            nc.vector.memset(h[:], 0.0)
            nc.vector.memset(m[:], 0.0)
            hT.append(h)
            mT.append(m)

        # ---- recurrence ----
        n_chunks = t_steps // ch
        xw_tiles = {}

        def emit_gather(c):
            xw = gpool.tile([128, 4, nidx_ch], f16, tag="xw")
            nc.gpsimd.dma_gather(
                xw[:],
                table[:],
                idx_sb[:, c * (nidx_ch // 16) : (c + 1) * (nidx_ch // 16)],
                nidx_ch,
                nidx_ch,
                G4,
                transpose=True,
                single_packet=False,
                sbuf_tokens_per_rank=128,
                sbuf_free_dim_per_rank=G4 * 2,  # 1024 B: one full row per rank stripe
            )
            xw_tiles[c] = xw

        emit_gather(0)
        for c in range(n_chunks):
            if c + 1 < n_chunks:
                emit_gather(c + 1)
            xw = xw_tiles.pop(c)
            for k in range(ch):
                toff = k * BC
                ps_cur = []
                # 5-matmul PSUM group per sub-block: gate matmuls (first one
                # start=True zeroes the full-bank tile) then the xw injection
                # last.  Keeping the xw matmul inside the group makes the
                # scheduler place it with the gates instead of deferring it
                # behind a coarse Activation-semaphore wait.
                for s in range(S):
                    bs = BSL[s]
                    ps_full = psg.tile([128, 512], f32, tag=f"ps{s}")
                    ps = ps_full[:, 0 : 4 * bs]
                    ps_cur.append(ps)
                    for gb in range(4):
                        nc.tensor.matmul(
                            ps[:, gb * bs : (gb + 1) * bs],
                            whh_sb[:, gb * HID : (gb + 1) * HID],
                            hT[s][:],
                            start=(gb == 0),
                            stop=False,
                            skip_group_check=True,
                        )
                    nc.tensor.matmul(
                        ps[:],
                        id_sb[:],
                        xw[:, :, toff + BOFF[s] : toff + BOFF[s] + bs],
                        start=False,
                        stop=True,
                        skip_group_check=True,
                    )
                # elementwise cell per sub-block
                for s in range(S):
                    bs = BSL[s]
                    ps = ps_cur[s]
                    # ONE tanh over [f,g,i]: tf = tanh(zf/2) etc (host-halved)
                    sg3 = spool.tile([128, 3 * bs], f16, tag=f"sg3{s}")
                    nc.scalar.activation(sg3[:], ps[:, 0 : 3 * bs], AF.Tanh)
                    # o-gate tanh: only needed by the last DVE op of the step
                    to = spool.tile([128, bs], f16, tag=f"to{s}")
                    nc.scalar.activation(to[:], ps[:, 3 * bs : 4 * bs], AF.Tanh)
                    # u1 = (tf+1)*m  = 2f * m          (critical path)
                    u1 = spool.tile([128, bs], f32, tag=f"u1{s}")
                    nc.vector.scalar_tensor_tensor(
                        u1[:], sg3[:, 0:bs], 1.0, mT[s][:], Alu.add, Alu.mult
                    )
                    # u2 = (ti+1)*tg = 2i * g          (off critical path)
                    u2 = spool.tile([128, bs], f16, tag=f"u2{s}")
                    nc.vector.scalar_tensor_tensor(
                        u2[:], sg3[:, 2 * bs : 3 * bs], 1.0,
                        sg3[:, bs : 2 * bs], Alu.add, Alu.mult,
                    )
                    # m' = 0.5*u1 + u2  (= 2c')
                    nc.vector.scalar_tensor_tensor(
                        mT[s][:], u1[:], 0.5, u2[:], Alu.mult, Alu.add
                    )
                    # h~' = (to+1) * y*(1 + RC*y^2), y = clamp(m', +-M0)
                    nc.vector._custom_dve(
                        hop, out=hT[s][:], in0=to[:], in1=mT[s][:],
                        s0=float(M0), s1=float(RC),
                    )

        # ---- classifier ----
        hall = spool.tile([HID, BC], f16, tag="hall")
        for s in range(S):
            nc.vector.tensor_copy(
                hall[:, BOFF[s] : BOFF[s] + BSL[s]], hT[s][:]
            )
        pc = ptab.tile([BC, VOCAB], f32, tag="pcls")
        nc.tensor.matmul(
            pc[:],
            hall[:],
            wcls_sb[:],
            start=True,
            stop=False,
            skip_group_check=True,
        )
        nc.tensor.matmul(
            pc[:],
            ones_sb[:],
            bcls_sb[:],
            start=False,
            stop=True,
            skip_group_check=True,
        )
        out_sb = spool.tile([BC, VOCAB], f32, tag="out")
        nc.vector.tensor_copy(out_sb[:], pc[:])
        nc.sync.dma_start(out_d[:], out_sb[:])

    nc.compile()
    return nc


def prep_inputs(inputs, emb, W_ih, W_hh, b_ih, b_hh, W_cls, b_cls, t_steps=T):
    """Host-side input marshaling: gate reorder [f,g,i,o], sigmoid-via-tanh
    pre-halving, h~ = 2h/A1 state-scale folding, transposes, vocab interleave
    permutation, and per-core token index wrap."""
    # torch gate order in rows: i [0:H], f [H:2H], g [2H:3H], o [3H:4H]
    perm = np.concatenate(
        [np.arange(128, 256), np.arange(256, 384), np.arange(0, 128),
         np.arange(384, 512)]
    )  # -> [f, g, i, o]
    # i/f/o pre-activations halved (sigmoid(z) = (tanh(z/2)+1)/2); g unscaled.
    blk = np.concatenate(
        [np.full(HID, 0.5), np.full(HID, 1.0), np.full(HID, 0.5),
         np.full(HID, 0.5)]
    ).astype(np.float32)

    Wih_r = np.asarray(W_ih, np.float32)[perm] * blk[:, None]
    bias_r = (np.asarray(b_ih, np.float32) + np.asarray(b_hh, np.float32))[perm] * blk
    # W_hh additionally absorbs the h~ = 2h/A1 state scale: h = 0.5*A1*h~.
    Whh_r = np.asarray(W_hh, np.float32)[perm] * blk[:, None] * (0.5 * A1)

    # fused token table: row v = emb[v] @ Wih_r.T + bias_r, laid out for the
    # SBUF-source dma_gather (partition v%128, free stripe v//128)
    tab = (np.asarray(emb, np.float32) @ Wih_r.T + bias_r).astype(np.float16)
    tab = np.ascontiguousarray(
        tab.reshape(2, 128, G4).transpose(1, 0, 2).reshape(128, 2 * G4)
    )

    common = {
        "table": tab,
        "whh": np.ascontiguousarray(Whh_r.T).astype(np.float16),
        "ident": np.eye(HID, dtype=np.float16),
        "wcls": np.ascontiguousarray(
            (0.5 * A1) * np.asarray(W_cls, np.float32).T
        ).astype(np.float16),
        "bcls": np.asarray(b_cls, np.float32)[None, :].astype(np.float16),
    }

    tok = np.asarray(inputs)
    in_maps = []
    for cidx in range(N_CORES):
        tc_ = tok[cidx * BC : (cidx + 1) * BC, :t_steps]  # [64, t]
        flat = tc_.T.reshape(-1).astype(np.int16)  # t-major: idx j = t*64 + b
        wrapped = flat.reshape(-1, 16).T  # [16, n/16]; idx j at [j%16, j//16]
        idxs = np.ascontiguousarray(np.tile(wrapped, (8, 1)))  # [128, n/16]
        m = dict(common)
        m["idxs"] = idxs
        in_maps.append(m)
    return in_maps


_NC_CACHE = {}


def kernel(inputs, emb, W_ih, W_hh, b_ih, b_hh, W_cls, b_cls):
    import concourse.bass_utils as bass_utils

    if "nc" not in _NC_CACHE:
        _NC_CACHE["nc"] = build_kernel()
    nc = _NC_CACHE["nc"]
    in_maps = prep_inputs(inputs, emb, W_ih, W_hh, b_ih, b_hh, W_cls, b_cls)
    res = bass_utils.run_bass_kernel_spmd(
        nc, in_maps, core_ids=list(range(N_CORES))
    )
    out = np.concatenate([r["out"] for r in res.results], axis=0)
    return np.ascontiguousarray(out.astype(np.float32))


# revision 30
# speedup vs baseline: 1.0067x; 1.0032x over previous
"""CharLSTM Trainium2 kernel.

Model: tokens [512, 512] -> emb gather -> xw = x @ W_ih.T + biases
-> 512-step LSTM recurrence -> h_last @ W_cls.T + b_cls -> [512, 256] logits.

Strategy:
  * Data-parallel over batch: 8 cores x 64 sequences each.  Weights replicated.
  * Per core, state kept transposed: hT/mT = [128 (hid), 64 (batch)] split into
    S=2 pipelined sub-blocks of 32 so PE/ACT/DVE overlap across the serial
    T=512 chain.
  * The embedding + input projection + biases are fused on the HOST into one
    256x512 token table (row v = emb[v] @ W_ih_r.T + b; pure weight
    preprocessing), DMA'd to SBUF once, and gathered per 2-step chunk with
    gpsimd dma_gather(transpose=True), which lands xwT directly in the
    [gate-dims, tokens] orientation the recurrence needs.  This avoids
    materializing the 256 MB xw tensor in HBM.
  * Gate columns are reordered to [f, g, i, o] and the i/f/o pre-activations
    are halved on the host so sigmoid(z) = (tanh(z/2)+1)/2 comes out of ONE
    Tanh activation over [f,g,i]; the o-gate gets its own small Tanh (it is
    off the critical path until the very end of the step).
  * The c-state is kept as m = 2c.  The per-step serial chain is then:
      PE gates -> ACT tanh[f,g,i] -> DVE u1=(tf+1)*m -> DVE m'=0.5*u1+u2
      -> DVE h' (custom fused op) -> PE next gates
    where u2=(ti+1)*tg runs off-chain.  tanh(c') for the hidden update is a
    clamped cubic evaluated INSIDE a custom 8-stage DVE instruction
    (LSTM_HGATE_ANT, registered at runtime via the dve_ops extension point)
    fused with the output gate:
      h~ = (to+1) * y*(1 + RC*y^2),  y = clamp(m', +-M0)
    The cubic's leading coefficient A1 and the 1/2 factors are folded into
    W_hh / W_cls on the host (stored hidden state is h~ = 2h/A1).  This
    removes the second Activation-engine visit (tanh(c)) and one DVE stage
    from the serial chain: modeled step time drops ~2030ns -> ~1564ns
    (1057832 ns -> 813529 ns total; rel err vs fp64 reference ~4.0e-3,
    verified on hardware).  Constant DMAs are issued in dependency order
    (idxs/table feed the first gather) to shave the prologue.
"""

import os
import sys
from contextlib import ExitStack

import numpy as np

for _p in ("/opt/trn_rl_repo", "/opt/pypackages"):
    if _p not in sys.path and os.path.isdir(_p):
        sys.path.append(_p)

VOCAB, EMB, HID = 256, 32, 128
B, T = 512, 512
N_CORES = 8
BC = B // N_CORES  # 64 sequences per core
BSL = [32, 32]     # pipelined sub-block sizes per core (sum = BC)
S = len(BSL)
BOFF = [sum(BSL[:i]) for i in range(S)]  # batch offsets of each sub-block
PSG_BUFS = 2       # PSUM tiles in flight per sub-block
G4 = 4 * HID       # 512 gate dims

# clamped-cubic tanh(c) approximation (c tracked as m = 2c):
#   tanh(m/2) ~= A1 * y * (1 + RC*y^2),  y = clamp(m, +-M0)
# Fit of tanh(m/2) on [0, M0]; end-to-end logit rel-err vs exact ~4e-3.
M0 = 1.6
A1 = 0.4951799
RC = -0.0649799

# o-gate: tanh(zo/2) ~= P1O*y + P3O*y^3, y = clamp(zo/2, +-X0O) (cubic sigma)
X0O = 1.1
P1O = 0.972540
P3O = -0.211690

_HOP_CACHE = {}


def _register_one(name, spec):
    """Register one custom DVE op (idempotent); returns the DveOp."""
    import concourse.dve_ops as dve_ops
    from concourse.dve_spec import _has_src1, lower
    from concourse.dve_uop import DveOpSpec

    for op in dve_ops.OPS:
        if op.name == name:
            return op
    row = dve_ops._CUSTOM_DVE_ROW_BASE + len(dve_ops.OPS)
    shas = {}
    for ver in ("v3", "v4"):
        try:
            u = lower(spec, ver=ver)
            shas[ver] = DveOpSpec(
                name=name, opcode=row, uops=u, rd1_en=_has_src1(spec)
            ).sha(ver)
        except Exception:
            pass
    op = dve_ops.DveOp(name, spec, subdim=False, uops_sha=shas)
    dve_ops.OPS.append(op)
    dve_ops.CUSTOM_DVE_SPECS[op.name] = spec
    dve_ops._SUB_OPCODE_FOR_NAME[op.name] = row
    return op


def _register_custom_ops():
    """Register the fused LSTM custom DVE ops (idempotent)."""
    if "ops" in _HOP_CACHE:
        return _HOP_CACHE["ops"]
    from concourse.dve_spec import (
        C0,
        C1,
        C2,
        One,
        Spec,
        Src0,
        Src1,
        Zero,
        maxx,
        minn,
        sq,
    )

    # h~ = (to + 1) * y*(1 + C1*y^2), y = clamp(m', +-C0)
    y = maxx(minn(Src1, C0), Zero - C0)
    hbody = (Src0 + One) * (y * (sq(y) * C1 + One))

    def _href(in0, in1, s0, s1, imm2):
        yy = np.clip(in1.astype(np.float32), -s0, s0)
        return (
            (in0.astype(np.float32) + 1.0) * (yy * (yy * yy * s1 + 1.0))
        ).astype(np.float32)

    hop = _register_one("LSTM_HGATE_ANT", Spec(body=hbody, reference=_href))

    # to = C2*y + C1*y^3, y = clamp(zo, +-C0): cubic tanh for the o-gate,
    # read straight from PSUM (zo is pre-halved on the host)
    yo = maxx(minn(Src0, C0), Zero - C0)
    tbody = (sq(yo) * C1 + C2) * yo

    def _tref(in0, in1, s0, s1, imm2):
        yy = np.clip(in0.astype(np.float32), -s0, s0)
        return ((yy * yy * s1 + imm2) * yy).astype(np.float32)

    top = _register_one("LSTM_TO_ANT", Spec(body=tbody, reference=_tref))
    _HOP_CACHE["ops"] = (hop, top)
    return hop, top


def build_kernel(t_steps=T, ch=2, debug=False):
    """Build + compile the per-core SPMD program. Returns the Bacc object."""
    import concourse.bacc as bacc
    import concourse.bass as bass
    import concourse.mybir as mybir
    import concourse.tile as tile

    hop, top = _register_custom_ops()

    dt = mybir.dt
    AF = mybir.ActivationFunctionType
    Alu = mybir.AluOpType
    f32, f16, i16 = dt.float32, dt.float16, dt.int16

    assert t_steps % ch == 0
    nidx_ch = ch * BC          # gathered tokens per chunk
    assert nidx_ch % 128 == 0

    nc = bacc.Bacc(
        "TRN2",
        target_bir_lowering=False,
        debug=debug,
        num_devices=N_CORES,
    )

    # ---- I/O ----
    table_d = nc.dram_tensor("table", [128, 2 * G4], f16, kind="ExternalInput")
    whh_d = nc.dram_tensor("whh", [HID, G4], f16, kind="ExternalInput")
    id_d = nc.dram_tensor("ident", [HID, HID], f16, kind="ExternalInput")
    wcls_d = nc.dram_tensor("wcls", [HID, VOCAB], f16, kind="ExternalInput")
    bcls_d = nc.dram_tensor("bcls", [1, VOCAB], f16, kind="ExternalInput")
    idxs_d = nc.dram_tensor(
        "idxs", [128, t_steps * BC // 16], i16, kind="ExternalInput"
    )
    out_d = nc.dram_tensor("out", [BC, VOCAB], f32, kind="ExternalOutput")

    with tile.TileContext(nc) as tc, ExitStack() as ctx:
        const = ctx.enter_context(tc.tile_pool(name="const", bufs=1))
        ptab = ctx.enter_context(
            tc.tile_pool(name="ptab", bufs=1, space=bass.MemorySpace.PSUM)
        )
        psg = ctx.enter_context(
            tc.tile_pool(name="psg", bufs=PSG_BUFS, space=bass.MemorySpace.PSUM)
        )
        gpool = ctx.enter_context(tc.tile_pool(name="gpool", bufs=3))
        spool = ctx.enter_context(tc.tile_pool(name="spool", bufs=8))

        # ---- load constants ----
        # DMA issue order matters: each dma_start costs ~565ns of serial SP
        # sequencer time, and the first gather waits on idxs+table, step 0 on
        # ident+whh.  Issue in dependency order; wcls/bcls are only needed at
        # the very end.
        idx_sb = const.tile([128, t_steps * BC // 16], i16, tag="idxs")
        nc.sync.dma_start(idx_sb[:], idxs_d[:])

        # ---- fused token table (built on host from emb/W_ih/biases) ----
        # table row (vocab v) holds [f | g | i | o] pre-activations incl. both
        # biases (i/f/o halved for the sigmoid-via-tanh trick).  SBUF layout is
        # rank-packed for dma_gather's SBUF-source mode: vocab row v lives at
        # partition v %% 128, fp16 columns [(v // 128)*512, (v // 128 + 1)*512).
        table = const.tile([128, 2 * G4], f16, tag="table")
        nc.sync.dma_start(table[:], table_d[:])
        id_sb = const.tile([HID, HID], f16, tag="ident")
        nc.sync.dma_start(id_sb[:], id_d[:])
        whh_sb = const.tile([HID, G4], f16, tag="whh")
        nc.sync.dma_start(whh_sb[:], whh_d[:])
        wcls_sb = const.tile([HID, VOCAB], f16, tag="wcls")
        nc.sync.dma_start(wcls_sb[:], wcls_d[:])
        bcls_sb = const.tile([1, VOCAB], f16, tag="bcls")
        nc.sync.dma_start(bcls_sb[:], bcls_d[:])

        ones_sb = const.tile([1, BC], f16, tag="ones")
        nc.vector.memset(ones_sb[:], 1.0)

        # ---- state ----
        hT = []
        mT = []
        for s in range(S):
            h = const.tile([HID, BSL[s]], f16, tag=f"h{s}")
            m = const.tile([HID, BSL[s]], f32, tag=f"m{s}")
            nc.vector.memset(h[:], 0.0)
            nc.vector.memset(m[:], 0.0)
            hT.append(h)
            mT.append(m)

        # ---- recurrence ----
        n_chunks = t_steps // ch
        xw_tiles = {}

        def emit_gather(c):
            xw = gpool.tile([128, 4, nidx_ch], f16, tag="xw")
            nc.gpsimd.dma_gather(
                xw[:],
                table[:],
                idx_sb[:, c * (nidx_ch // 16) : (c + 1) * (nidx_ch // 16)],
                nidx_ch,
                nidx_ch,
                G4,
                transpose=True,
                single_packet=False,
                sbuf_tokens_per_rank=128,
                sbuf_free_dim_per_rank=G4 * 2,  # 1024 B: one full row per rank stripe
            )
            xw_tiles[c] = xw

        emit_gather(0)
        for c in range(n_chunks):
            if c + 1 < n_chunks:
                emit_gather(c + 1)
            xw = xw_tiles.pop(c)
            for k in range(ch):
                toff = k * BC
                ps_cur = []
                # 5-matmul PSUM group per sub-block: gate matmuls (first one
                # start=True zeroes the full-bank tile) then the xw injection
                # last.  Keeping the xw matmul inside the group makes the
                # scheduler place it with the gates instead of deferring it
                # behind a coarse Activation-semaphore wait.
                for s in range(S):
                    bs = BSL[s]
                    ps_full = psg.tile([128, 512], f32, tag=f"ps{s}")
                    ps = ps_full[:, 0 : 4 * bs]
                    ps_cur.append(ps)
                    for gb in range(4):
                        nc.tensor.matmul(
                            ps[:, gb * bs : (gb + 1) * bs],
                            whh_sb[:, gb * HID : (gb + 1) * HID],
                            hT[s][:],
                            start=(gb == 0),
                            stop=False,
                            skip_group_check=True,
                        )
                    nc.tensor.matmul(
                        ps[:],
                        id_sb[:],
                        xw[:, :, toff + BOFF[s] : toff + BOFF[s] + bs],
                        start=False,
                        stop=True,
                        skip_group_check=True,
                    )
                # elementwise cell per sub-block
                for s in range(S):
                    bs = BSL[s]
                    ps = ps_cur[s]
                    # ONE tanh over [f,g,i]: tf = tanh(zf/2) etc (host-halved)
                    sg3 = spool.tile([128, 3 * bs], f16, tag=f"sg3{s}")
                    nc.scalar.activation(sg3[:], ps[:, 0 : 3 * bs], AF.Tanh)
                    # o-gate tanh: only needed by the last DVE op of the step
                    to = spool.tile([128, bs], f16, tag=f"to{s}")
                    nc.scalar.activation(to[:], ps[:, 3 * bs : 4 * bs], AF.Tanh)
                    # u1 = (tf+1)*m  = 2f * m          (critical path)
                    u1 = spool.tile([128, bs], f32, tag=f"u1{s}")
                    nc.vector.scalar_tensor_tensor(
                        u1[:], sg3[:, 0:bs], 1.0, mT[s][:], Alu.add, Alu.mult
                    )
                    # u2 = (ti+1)*tg = 2i * g          (off critical path)
                    u2 = spool.tile([128, bs], f16, tag=f"u2{s}")
                    nc.vector.scalar_tensor_tensor(
                        u2[:], sg3[:, 2 * bs : 3 * bs], 1.0,
                        sg3[:, bs : 2 * bs], Alu.add, Alu.mult,
                    )
                    # m' = 0.5*u1 + u2  (= 2c')
                    nc.vector.scalar_tensor_tensor(
                        mT[s][:], u1[:], 0.5, u2[:], Alu.mult, Alu.add
                    )
                    # h~' = (to+1) * y*(1 + RC*y^2), y = clamp(m', +-M0)
                    nc.vector._custom_dve(
                        hop, out=hT[s][:], in0=to[:], in1=mT[s][:],
                        s0=float(M0), s1=float(RC),
                    )

        # ---- classifier ----
        hall = spool.tile([HID, BC], f16, tag="hall")
        for s in range(S):
            nc.vector.tensor_copy(
                hall[:, BOFF[s] : BOFF[s] + BSL[s]], hT[s][:]
            )
        pc = ptab.tile([BC, VOCAB], f32, tag="pcls")
        nc.tensor.matmul(
            pc[:],
            hall[:],
            wcls_sb[:],
            start=True,
            stop=False,
            skip_group_check=True,
        )
        nc.tensor.matmul(
            pc[:],
            ones_sb[:],
            bcls_sb[:],
            start=False,
            stop=True,
            skip_group_check=True,
        )
        out_sb = spool.tile([BC, VOCAB], f32, tag="out")
        nc.vector.tensor_copy(out_sb[:], pc[:])
        nc.sync.dma_start(out_d[:], out_sb[:])

    nc.compile()
    return nc


def prep_inputs(inputs, emb, W_ih, W_hh, b_ih, b_hh, W_cls, b_cls, t_steps=T):
    """Host-side input marshaling: gate reorder [f,g,i,o], sigmoid-via-tanh
    pre-halving, h~ = 2h/A1 state-scale folding, transposes, vocab interleave
    permutation, and per-core token index wrap."""
    # torch gate order in rows: i [0:H], f [H:2H], g [2H:3H], o [3H:4H]
    perm = np.concatenate(
        [np.arange(128, 256), np.arange(256, 384), np.arange(0, 128),
         np.arange(384, 512)]
    )  # -> [f, g, i, o]
    # i/f/o pre-activations halved (sigmoid(z) = (tanh(z/2)+1)/2); g unscaled.
    blk = np.concatenate(
        [np.full(HID, 0.5), np.full(HID, 1.0), np.full(HID, 0.5),
         np.full(HID, 0.5)]
    ).astype(np.float32)

    Wih_r = np.asarray(W_ih, np.float32)[perm] * blk[:, None]
    bias_r = (np.asarray(b_ih, np.float32) + np.asarray(b_hh, np.float32))[perm] * blk
    # W_hh additionally absorbs the h~ = 2h/A1 state scale: h = 0.5*A1*h~.
    Whh_r = np.asarray(W_hh, np.float32)[perm] * blk[:, None] * (0.5 * A1)

    # fused token table: row v = emb[v] @ Wih_r.T + bias_r, laid out for the
    # SBUF-source dma_gather (partition v%128, free stripe v//128)
    tab = (np.asarray(emb, np.float32) @ Wih_r.T + bias_r).astype(np.float16)
    tab = np.ascontiguousarray(
        tab.reshape(2, 128, G4).transpose(1, 0, 2).reshape(128, 2 * G4)
    )

    common = {
        "table": tab,
        "whh": np.ascontiguousarray(Whh_r.T).astype(np.float16),
        "ident": np.eye(HID, dtype=np.float16),
        "wcls": np.ascontiguousarray(
            (0.5 * A1) * np.asarray(W_cls, np.float32).T
        ).astype(np.float16),
        "bcls": np.asarray(b_cls, np.float32)[None, :].astype(np.float16),
    }

    tok = np.asarray(inputs)
    in_maps = []
    for cidx in range(N_CORES):
        tc_ = tok[cidx * BC : (cidx + 1) * BC, :t_steps]  # [64, t]
        flat = tc_.T.reshape(-1).astype(np.int16)  # t-major: idx j = t*64 + b
        wrapped = flat.reshape(-1, 16).T  # [16, n/16]; idx j at [j%16, j//16]
        idxs = np.ascontiguousarray(np.tile(wrapped, (8, 1)))  # [128, n/16]
        m = dict(common)
        m["idxs"] = idxs
        in_maps.append(m)
    return in_maps


_NC_CACHE = {}


def kernel(inputs, emb, W_ih, W_hh, b_ih, b_hh, W_cls, b_cls):
    import concourse.bass_utils as bass_utils

    if "nc" not in _NC_CACHE:
        _NC_CACHE["nc"] = build_kernel()
    nc = _NC_CACHE["nc"]
    in_maps = prep_inputs(inputs, emb, W_ih, W_hh, b_ih, b_hh, W_cls, b_cls)
    res = bass_utils.run_bass_kernel_spmd(
        nc, in_maps, core_ids=list(range(N_CORES))
    )
    out = np.concatenate([r["out"] for r in res.results], axis=0)
    return np.ascontiguousarray(out.astype(np.float32))


# revision 43
# speedup vs baseline: 1.0230x; 1.0162x over previous
"""CharLSTM Trainium2 kernel.

Model: tokens [512, 512] -> emb gather -> xw = x @ W_ih.T + biases
-> 512-step LSTM recurrence -> h_last @ W_cls.T + b_cls -> [512, 256] logits.

Strategy:
  * Data-parallel over batch: 8 cores x 64 sequences each.  Weights replicated.
  * Per core, state kept transposed: hT/mT = [128 (hid), 64 (batch)] split into
    S=2 pipelined sub-blocks of 32 so PE/ACT/DVE overlap across the serial
    T=512 chain.
  * The embedding + input projection + biases are fused on the HOST into one
    256x512 token table (row v = emb[v] @ W_ih_r.T + b; pure weight
    preprocessing), DMA'd to SBUF once, and gathered per 2-step chunk with
    gpsimd dma_gather(transpose=True), which lands xwT directly in the
    [gate-dims, tokens] orientation the recurrence needs.  This avoids
    materializing the 256 MB xw tensor in HBM.
  * Gate columns are reordered to [f, g, i, o] and the i/f/o pre-activations
    are halved on the host so sigmoid(z) = (tanh(z/2)+1)/2 comes out of ONE
    Tanh activation over [f,g,i]; the o-gate gets its own small Tanh (it is
    off the critical path until the very end of the step).
  * The c-state is kept as m = 2c, stored f16 in the last quarter of a
    per-sub-block work tile [ti|tf|tg|m] so that BOTH gate products come out
    of ONE scalar_tensor_tensor: (in0+1)*in1 over the adjacent halves
    [ti,tf] x [tg,m] = [u2, u1].  The per-step serial chain is then:
      PE gates -> ACT tanh[i,f,g] -> DVE pair [u2,u1] -> DVE m'=0.5*u1+u2
      -> DVE h' (custom fused op) -> PE next gates
    tanh(c') for the hidden update is a clamped cubic evaluated INSIDE a
    custom 8-stage DVE instruction (LSTM_HGATE_ANT, registered at runtime
    via the dve_ops extension point) fused with the output gate:
      h~ = (to+1) * y*(1 + RC*y^2),  y = clamp(m', +-M0)
    The cubic's leading coefficient A1 and the 1/2 factors are folded into
    W_hh / W_cls on the host (stored hidden state is h~ = 2h/A1).  This
    removes the second Activation-engine visit (tanh(c)) and one DVE stage
    from the serial chain.  The xw injection runs FIRST in each PSUM group
    (start=True, no h dependency) so the PE executes it during the previous
    cell and the activation's stop-matmul wait shortens.  Modeled step time
    drops ~2030ns -> ~1539ns (1057832 ns -> 800585 ns total; rel err vs
    fp64 reference ~4.0e-3, verified on hardware).  Constant DMAs are
    issued in dependency order (idxs/table feed the first gather) to shave
    the prologue.
"""

import os
import sys
from contextlib import ExitStack

import numpy as np

for _p in ("/opt/trn_rl_repo", "/opt/pypackages"):
    if _p not in sys.path and os.path.isdir(_p):
        sys.path.append(_p)

VOCAB, EMB, HID = 256, 32, 128
B, T = 512, 512
N_CORES = 8
BC = B // N_CORES  # 64 sequences per core
BSL = [32, 32]     # pipelined sub-block sizes per core (sum = BC)
S = len(BSL)
BOFF = [sum(BSL[:i]) for i in range(S)]  # batch offsets of each sub-block
PSG_BUFS = 3       # PSUM tiles in flight per sub-block
G4 = 4 * HID       # 512 gate dims

# clamped-cubic tanh(c) approximation (c tracked as m = 2c):
#   tanh(m/2) ~= A1 * y * (1 + RC*y^2),  y = clamp(m, +-M0)
# Fit of tanh(m/2) on [0, M0]; end-to-end logit rel-err vs exact ~4e-3.
M0 = 1.6
A1 = 0.4951799
RC = -0.0649799

# o-gate: tanh(zo/2) ~= P1O*y + P3O*y^3, y = clamp(zo/2, +-X0O) (cubic sigma)
X0O = 1.1
P1O = 0.972540
P3O = -0.211690

_HOP_CACHE = {}


def _register_one(name, spec):
    """Register one custom DVE op (idempotent); returns the DveOp."""
    import concourse.dve_ops as dve_ops
    from concourse.dve_spec import _has_src1, lower
    from concourse.dve_uop import DveOpSpec

    for op in dve_ops.OPS:
        if op.name == name:
            return op
    row = dve_ops._CUSTOM_DVE_ROW_BASE + len(dve_ops.OPS)
    shas = {}
    for ver in ("v3", "v4"):
        try:
            u = lower(spec, ver=ver)
            shas[ver] = DveOpSpec(
                name=name, opcode=row, uops=u, rd1_en=_has_src1(spec)
            ).sha(ver)
        except Exception:
            pass
    op = dve_ops.DveOp(name, spec, subdim=False, uops_sha=shas)
    dve_ops.OPS.append(op)
    dve_ops.CUSTOM_DVE_SPECS[op.name] = spec
    dve_ops._SUB_OPCODE_FOR_NAME[op.name] = row
    return op


def _register_custom_ops():
    """Register the fused LSTM custom DVE ops (idempotent)."""
    if "ops" in _HOP_CACHE:
        return _HOP_CACHE["ops"]
    from concourse.dve_spec import (
        C0,
        C1,
        C2,
        One,
        Spec,
        Src0,
        Src1,
        Zero,
        maxx,
        minn,
        sq,
    )

    # h~ = (to + 1) * y*(1 + C1*y^2), y = clamp(m', +-C0)
    y = maxx(minn(Src1, C0), Zero - C0)
    hbody = (Src0 + One) * (y * (sq(y) * C1 + One))

    def _href(in0, in1, s0, s1, imm2):
        yy = np.clip(in1.astype(np.float32), -s0, s0)
        return (
            (in0.astype(np.float32) + 1.0) * (yy * (yy * yy * s1 + 1.0))
        ).astype(np.float32)

    hop = _register_one("LSTM_HGATE_ANT", Spec(body=hbody, reference=_href))

    # to = C2*y + C1*y^3, y = clamp(zo, +-C0): cubic tanh for the o-gate,
    # read straight from PSUM (zo is pre-halved on the host)
    yo = maxx(minn(Src0, C0), Zero - C0)
    tbody = (sq(yo) * C1 + C2) * yo

    def _tref(in0, in1, s0, s1, imm2):
        yy = np.clip(in0.astype(np.float32), -s0, s0)
        return ((yy * yy * s1 + imm2) * yy).astype(np.float32)

    top = _register_one("LSTM_TO_ANT", Spec(body=tbody, reference=_tref))
    _HOP_CACHE["ops"] = (hop, top)
    return hop, top


def build_kernel(t_steps=T, ch=2, debug=False):
    """Build + compile the per-core SPMD program. Returns the Bacc object."""
    import concourse.bacc as bacc
    import concourse.bass as bass
    import concourse.mybir as mybir
    import concourse.tile as tile

    hop, top = _register_custom_ops()

    dt = mybir.dt
    AF = mybir.ActivationFunctionType
    Alu = mybir.AluOpType
    f32, f16, i16 = dt.float32, dt.float16, dt.int16

    assert t_steps % ch == 0
    nidx_ch = ch * BC          # gathered tokens per chunk
    assert nidx_ch % 128 == 0

    nc = bacc.Bacc(
        "TRN2",
        target_bir_lowering=False,
        debug=debug,
        num_devices=N_CORES,
    )

    # ---- I/O ----
    table_d = nc.dram_tensor("table", [128, 2 * G4], f16, kind="ExternalInput")
    whh_d = nc.dram_tensor("whh", [HID, G4], f16, kind="ExternalInput")
    id_d = nc.dram_tensor("ident", [HID, HID], f16, kind="ExternalInput")
    wcls_d = nc.dram_tensor("wcls", [HID, VOCAB], f16, kind="ExternalInput")
    bcls_d = nc.dram_tensor("bcls", [1, VOCAB], f16, kind="ExternalInput")
    idxs_d = nc.dram_tensor(
        "idxs", [128, t_steps * BC // 16], i16, kind="ExternalInput"
    )
    out_d = nc.dram_tensor("out", [BC, VOCAB], f32, kind="ExternalOutput")

    with tile.TileContext(nc) as tc, ExitStack() as ctx:
        const = ctx.enter_context(tc.tile_pool(name="const", bufs=1))
        ptab = ctx.enter_context(
            tc.tile_pool(name="ptab", bufs=1, space=bass.MemorySpace.PSUM)
        )
        psg = ctx.enter_context(
            tc.tile_pool(name="psg", bufs=PSG_BUFS, space=bass.MemorySpace.PSUM)
        )
        gpool = ctx.enter_context(tc.tile_pool(name="gpool", bufs=3))
        spool = ctx.enter_context(tc.tile_pool(name="spool", bufs=8))

        # ---- load constants ----
        # DMA issue order matters: each dma_start costs ~565ns of serial SP
        # sequencer time, and the first gather waits on idxs+table, step 0 on
        # ident+whh.  Issue in dependency order; wcls/bcls are only needed at
        # the very end.
        idx_sb = const.tile([128, t_steps * BC // 16], i16, tag="idxs")
        nc.sync.dma_start(idx_sb[:], idxs_d[:])

        # ---- fused token table (built on host from emb/W_ih/biases) ----
        # table row (vocab v) holds [f | g | i | o] pre-activations incl. both
        # biases (i/f/o halved for the sigmoid-via-tanh trick).  SBUF layout is
        # rank-packed for dma_gather's SBUF-source mode: vocab row v lives at
        # partition v %% 128, fp16 columns [(v // 128)*512, (v // 128 + 1)*512).
        table = const.tile([128, 2 * G4], f16, tag="table")
        nc.sync.dma_start(table[:], table_d[:])
        id_sb = const.tile([HID, HID], f16, tag="ident")
        nc.sync.dma_start(id_sb[:], id_d[:])
        whh_sb = const.tile([HID, G4], f16, tag="whh")
        nc.sync.dma_start(whh_sb[:], whh_d[:])
        wcls_sb = const.tile([HID, VOCAB], f16, tag="wcls")
        nc.sync.dma_start(wcls_sb[:], wcls_d[:])
        bcls_sb = const.tile([1, VOCAB], f16, tag="bcls")
        nc.sync.dma_start(bcls_sb[:], bcls_d[:])

        ones_sb = const.tile([1, BC], f16, tag="ones")
        nc.vector.memset(ones_sb[:], 1.0)

        # ---- state ----
        # Per sub-block: hT (f16) and a combined work tile Wk = [ti|tf|tg|m]
        # (all f16, m = 2c in the last quarter).  The [i,f,g] tanh writes the
        # first three quarters; keeping m adjacent lets ONE stt compute both
        # gate products: (in0+1)*in1 over [ti,tf] x [tg,m] = [u2, u1].
        hT = []
        WK = []
        for s in range(S):
            h = const.tile([HID, BSL[s]], f16, tag=f"h{s}")
            w = const.tile([HID, 4 * BSL[s]], f16, tag=f"wk{s}")
            nc.vector.memset(h[:], 0.0)
            nc.vector.memset(w[:], 0.0)
            hT.append(h)
            WK.append(w)

        # ---- recurrence ----
        n_chunks = t_steps // ch
        xw_tiles = {}

        def emit_gather(c):
            xw = gpool.tile([128, 4, nidx_ch], f16, tag="xw")
            nc.gpsimd.dma_gather(
                xw[:],
                table[:],
                idx_sb[:, c * (nidx_ch // 16) : (c + 1) * (nidx_ch // 16)],
                nidx_ch,
                nidx_ch,
                G4,
                transpose=True,
                single_packet=False,
                sbuf_tokens_per_rank=128,
                sbuf_free_dim_per_rank=G4 * 2,  # 1024 B: one full row per rank stripe
            )
            xw_tiles[c] = xw

        emit_gather(0)
        for c in range(n_chunks):
            if c + 1 < n_chunks:
                emit_gather(c + 1)
            xw = xw_tiles.pop(c)
            for k in range(ch):
                toff = k * BC
                ps_cur = []
                # 5-matmul PSUM group per sub-block: gate matmuls (first one
                # start=True zeroes the full-bank tile) then the xw injection
                # last.  Keeping the xw matmul inside the group makes the
                # scheduler place it with the gates instead of deferring it
                # behind a coarse Activation-semaphore wait.
                for s in range(S):
                    bs = BSL[s]
                    ps_full = psg.tile([128, 512], f32, tag=f"ps{s}")
                    ps = ps_full[:, 0 : 4 * bs]
                    ps_cur.append(ps)
                    # xw first (start=True zeroes the bank): it has no h
                    # dependency, so the PE runs it during the previous cell
                    # and the activation's stop-matmul wait shortens.
                    nc.tensor.matmul(
                        ps[:],
                        id_sb[:],
                        xw[:, :, toff + BOFF[s] : toff + BOFF[s] + bs],
                        start=True,
                        stop=False,
                        skip_group_check=True,
                    )
                    for gb in range(4):
                        nc.tensor.matmul(
                            ps[:, gb * bs : (gb + 1) * bs],
                            whh_sb[:, gb * HID : (gb + 1) * HID],
                            hT[s][:],
                            start=False,
                            stop=(gb == 3),
                            skip_group_check=True,
                        )
                # elementwise cell per sub-block
                for s in range(S):
                    bs = BSL[s]
                    ps = ps_cur[s]
                    wk = WK[s]
                    # ONE tanh over [i,f,g] into the work tile [ti|tf|tg|m]
                    # (host-halved i/f: sigmoid-via-tanh); m = 2c (f16) sits
                    # in the last quarter, adjacent to tg
                    nc.scalar.activation(
                        wk[:, 0 : 3 * bs], ps[:, 0 : 3 * bs], AF.Tanh
                    )
                    # o-gate tanh: only needed by the last DVE op of the step
                    to = spool.tile([128, bs], f16, tag=f"to{s}")
                    nc.scalar.activation(to[:], ps[:, 3 * bs : 4 * bs], AF.Tanh)
                    # BOTH gate products in one stt: (in0+1)*in1 over the
                    # adjacent halves [ti,tf] x [tg,m] -> [u2, u1]
                    p2 = spool.tile([128, 2 * bs], f16, tag=f"p2{s}")
                    nc.vector.scalar_tensor_tensor(
                        p2[:], wk[:, 0 : 2 * bs], 1.0, wk[:, 2 * bs : 4 * bs],
                        Alu.add, Alu.mult,
                    )
                    # m' = 0.5*u1 + u2  (= 2c'), written back into the work tile
                    nc.vector.scalar_tensor_tensor(
                        wk[:, 3 * bs : 4 * bs], p2[:, bs : 2 * bs], 0.5,
                        p2[:, 0:bs], Alu.mult, Alu.add,
                    )
                    # h~' = (to+1) * y*(1 + RC*y^2), y = clamp(m', +-M0)
                    nc.vector._custom_dve(
                        hop, out=hT[s][:], in0=to[:],
                        in1=wk[:, 3 * bs : 4 * bs],
                        s0=float(M0), s1=float(RC),
                    )

        # ---- classifier ----
        hall = spool.tile([HID, BC], f16, tag="hall")
        for s in range(S):
            nc.vector.tensor_copy(
                hall[:, BOFF[s] : BOFF[s] + BSL[s]], hT[s][:]
            )
        pc = ptab.tile([BC, VOCAB], f32, tag="pcls")
        nc.tensor.matmul(
            pc[:],
            hall[:],
            wcls_sb[:],
            start=True,
            stop=False,
            skip_group_check=True,
        )
        nc.tensor.matmul(
            pc[:],
            ones_sb[:],
            bcls_sb[:],
            start=False,
            stop=True,
            skip_group_check=True,
        )
        out_sb = spool.tile([BC, VOCAB], f32, tag="out")
        nc.vector.tensor_copy(out_sb[:], pc[:])
        nc.sync.dma_start(out_d[:], out_sb[:])

    nc.compile()
    return nc


def prep_inputs(inputs, emb, W_ih, W_hh, b_ih, b_hh, W_cls, b_cls, t_steps=T):
    """Host-side input marshaling: gate reorder [f,g,i,o], sigmoid-via-tanh
    pre-halving, h~ = 2h/A1 state-scale folding, transposes, vocab interleave
    permutation, and per-core token index wrap."""
    # torch gate order in rows: i [0:H], f [H:2H], g [2H:3H], o [3H:4H] —
    # kept as-is (the kernel's PSUM block order is [i, f, g, o])
    perm = np.arange(4 * HID)
    # i/f/o pre-activations halved (sigmoid(z) = (tanh(z/2)+1)/2); g unscaled.
    blk = np.concatenate(
        [np.full(HID, 0.5), np.full(HID, 0.5), np.full(HID, 1.0),
         np.full(HID, 0.5)]
    ).astype(np.float32)

    Wih_r = np.asarray(W_ih, np.float32)[perm] * blk[:, None]
    bias_r = (np.asarray(b_ih, np.float32) + np.asarray(b_hh, np.float32))[perm] * blk
    # W_hh additionally absorbs the h~ = 2h/A1 state scale: h = 0.5*A1*h~.
    Whh_r = np.asarray(W_hh, np.float32)[perm] * blk[:, None] * (0.5 * A1)

    # fused token table: row v = emb[v] @ Wih_r.T + bias_r, laid out for the
    # SBUF-source dma_gather (partition v%128, free stripe v//128)
    tab = (np.asarray(emb, np.float32) @ Wih_r.T + bias_r).astype(np.float16)
    tab = np.ascontiguousarray(
        tab.reshape(2, 128, G4).transpose(1, 0, 2).reshape(128, 2 * G4)
    )

    common = {
        "table": tab,
        "whh": np.ascontiguousarray(Whh_r.T).astype(np.float16),
        "ident": np.eye(HID, dtype=np.float16),
        "wcls": np.ascontiguousarray(
            (0.5 * A1) * np.asarray(W_cls, np.float32).T
        ).astype(np.float16),
        "bcls": np.asarray(b_cls, np.float32)[None, :].astype(np.float16),
    }

    tok = np.asarray(inputs)
    in_maps = []
    for cidx in range(N_CORES):
        tc_ = tok[cidx * BC : (cidx + 1) * BC, :t_steps]  # [64, t]
        flat = tc_.T.reshape(-1).astype(np.int16)  # t-major: idx j = t*64 + b
        wrapped = flat.reshape(-1, 16).T  # [16, n/16]; idx j at [j%16, j//16]
        idxs = np.ascontiguousarray(np.tile(wrapped, (8, 1)))  # [128, n/16]
        m = dict(common)
        m["idxs"] = idxs
        in_maps.append(m)
    return in_maps


_NC_CACHE = {}


def kernel(inputs, emb, W_ih, W_hh, b_ih, b_hh, W_cls, b_cls):
    import concourse.bass_utils as bass_utils

    if "nc" not in _NC_CACHE:
        _NC_CACHE["nc"] = build_kernel()
    nc = _NC_CACHE["nc"]
    in_maps = prep_inputs(inputs, emb, W_ih, W_hh, b_ih, b_hh, W_cls, b_cls)
    res = bass_utils.run_bass_kernel_spmd(
        nc, in_maps, core_ids=list(range(N_CORES))
    )
    out = np.concatenate([r["out"] for r in res.results], axis=0)
    return np.ascontiguousarray(out.astype(np.float32))
